# revision 1
# baseline (speedup 1.0000x reference)
"""Bayer kernel-prediction demosaic network on 8 Trainium2 NeuronCores.

Pipeline per core (one batch x one 96-row band of the 374-row quarter-res
kernel grid, with halo):
  - conv0 (4->64) via im2col (K=36) matmul, relu
  - conv1..3 (64->64) as 6 tap-matmuls each (3 paired taps K=128 using a
    column-shifted duplicate of the activations in partitions 64:128,
    plus 3 single taps K=64), relu
  - conv4 (64->490) same 6-matmul structure, 4 output-channel blocks,
    per-output-row tiles; exp(.) applied on PSUM eviction (ScalarE) with
    the conv bias folded into the activation bias -> E (bf16)
  - patch tensor Prep[tap_channel, pixel] gathered straight from the
    (padded) phase planes in DRAM with strided DMAs (bf16)
  - M = E * Prep on VectorE
  - numerator / denominator = group-mask matmuls (bf16) contracting the
    490 tap-channels -> [8, pix] in PSUM; out = num * reciprocal(den)
All conv-chain matmuls run in fp16 (full-rate, FWL weight loads) with
fp32 PSUM accumulation.
Host does phase extraction, weight re-layout, sharding w/ halo, and the
final pixel-shuffle assembly.
"""

import sys

sys.path.insert(0, "/opt/trn_rl_repo")

import numpy as np
import ml_dtypes

# ---------------- geometry constants ----------------
KS = 7
K2 = 49
BS = 2
H = W = 768
QH = QW = 384          # quarter-res
KR_TOT = 374           # valid kernel rows/cols
BANDS = 4              # bands per batch -> 8 cores
KR = 96                # kernel rows computed per core (94/92 valid)
RB = 16                # kernel rows per conv block
SRB = 8                # kernel rows per apply sub-block
NBLK = KR // RB
GW = 386               # conv grid width (L buffers, x36)
EW = 376               # apply/kernel grid width (374 valid + 2)
XW = 388               # x slab width (384 data + 4 zero)
XR = 106               # x slab rows (96 + 10)
ROWS0, ROWS1, ROWS2, ROWS3 = 24, 22, 20, 18   # conv0..conv3 out rows/block
F36 = ROWS0 * GW
F0, F1, F2, F3 = ROWS0 * GW, ROWS1 * GW, ROWS2 * GW, ROWS3 * GW
FE = SRB * EW          # flat apply pixels per sub-block
OUTF = KR * EW
MBLK = [0, 128, 256, 384, 490]     # channel block boundaries
# plane (x-slab channel) feeding each 49-tap chunk of the 490 kernels:
# x channels: 0=g0 1=b 2=r 3=g1 ; chunks: 3x red, 3x blue, (g0,g1)x2
CHUNK_PLANE = [2, 2, 2, 1, 1, 1, 0, 3, 0, 3]
# 49-chunk -> output group (greens pair up)
CHUNK_GROUP = [0, 1, 2, 3, 4, 5, 6, 6, 7, 7]

TRACE = False          # set True (module attr) to profile the run
LAST_EXEC_NS = None
LAST_RESULTS = None

_cache = {}


def _build():
    import concourse.bass as bass
    import concourse.bacc as bacc
    import concourse.mybir as mybir
    import concourse.tile as tile

    f32 = mybir.dt.float32
    f16 = mybir.dt.float16
    bf16 = mybir.dt.bfloat16
    AF = mybir.ActivationFunctionType
    ALU = mybir.AluOpType

    nc = bacc.Bacc("TRN2", target_bir_lowering=False, debug=False,
                   enable_asserts=False)

    xs = nc.dram_tensor("xs", [4, XR, XW], f16, kind="ExternalInput")
    xg = nc.dram_tensor("xg", [490, KR, EW], bf16, kind="ExternalInput")
    w0p = nc.dram_tensor("w0p", [36, 64], f16, kind="ExternalInput")
    wp = nc.dram_tensor("wp", [128, 9, 64], f16, kind="ExternalInput")
    ws = nc.dram_tensor("ws", [64, 9, 64], f16, kind="ExternalInput")
    w4p = nc.dram_tensor("w4p", [128, 3, 490], f16, kind="ExternalInput")
    w4s = nc.dram_tensor("w4s", [64, 3, 490], f16, kind="ExternalInput")
    b03 = nc.dram_tensor("b03", [64, 4], f32, kind="ExternalInput")
    b4 = nc.dram_tensor("b4", [128, 4], f32, kind="ExternalInput")
    gm = nc.dram_tensor("gm", [128, 4, 8], bf16, kind="ExternalInput")
    out = nc.dram_tensor("out", [8, OUTF], f32, kind="ExternalOutput")

    def ntiles(total, tsz=512):
        o = 0
        while o < total:
            n = min(tsz, total - o)
            yield o, n
            o += n

    with tile.TileContext(nc) as tc:
        with (
            tc.tile_pool(name="wts", bufs=1) as wts,
            tc.tile_pool(name="big", bufs=1) as big,
            tc.tile_pool(name="sm", bufs=2) as sm,
            tc.tile_pool(name="pscv", bufs=4, space="PSUM") as pscv,
            tc.tile_pool(name="psd", bufs=2, space="PSUM") as psd,
            tc.tile_pool(name="psn", bufs=2, space="PSUM") as psn,
        ):
            w0p_sb = wts.tile([36, 64], f16)
            wp_sb = wts.tile([128, 9, 64], f16)
            ws_sb = wts.tile([64, 9, 64], f16)
            w4p_sb = wts.tile([128, 3, 490], f16)
            w4s_sb = wts.tile([64, 3, 490], f16)
            b03_sb = wts.tile([64, 4], f32)
            b4_sb = wts.tile([128, 4], f32)
            gm_sb = wts.tile([128, 4, 8], bf16)
            for dst, src in ((w0p_sb, w0p), (wp_sb, wp), (ws_sb, ws),
                             (w4p_sb, w4p), (w4s_sb, w4s), (b03_sb, b03),
                             (b4_sb, b4), (gm_sb, gm)):
                nc.sync.dma_start(dst[:], src.ap())

            NS = RB // SRB

            def convtile(Lprev, ps, li, o, n):
                """conv1..3 tile: 6 tap matmuls as 3 rounds of 2 concurrent
                column tiles (left half {pair0, pair2, single1}, right half
                {pair1, single0, single2})."""
                mms = [
                    (0, wp_sb[:, 3 * li + 0, :], Lprev[0:128, o: o + n]),
                    (1, wp_sb[:, 3 * li + 1, :],
                     Lprev[0:128, o + GW: o + GW + n]),
                    (0, wp_sb[:, 3 * li + 2, :],
                     Lprev[0:128, o + 2 * GW: o + 2 * GW + n]),
                    (1, ws_sb[:, 3 * li + 0, :], Lprev[0:64, o + 2: o + 2 + n]),
                    (0, ws_sb[:, 3 * li + 1, :],
                     Lprev[0:64, o + GW + 2: o + GW + 2 + n]),
                    (1, ws_sb[:, 3 * li + 2, :],
                     Lprev[0:64, o + 2 * GW + 2: o + 2 * GW + 2 + n]),
                ]
                import os as _os
                if _os.environ.get("CONVTILE", "1") == "1":
                    seen = [False, False]
                    for idx, (h, lhsT, rhs) in enumerate(mms):
                        nc.tensor.matmul(ps[64 * h:64 * h + 64, 0:n], lhsT,
                                         rhs, start=not seen[h], stop=idx >= 4,
                                         tile_position=(0, 64 * h),
                                         skip_group_check=True)
                        seen[h] = True
                else:
                    for idx, (h, lhsT, rhs) in enumerate(mms):
                        nc.tensor.matmul(ps[0:64, 0:n], lhsT, rhs,
                                         start=idx == 0, stop=idx == 5)

            def ndpairs():
                """(o, n, o2, n2) chunk pairs covering [0, FE)."""
                import os as _os
                chunks = list(ntiles(FE))
                if _os.environ.get("NDPACK", "0") == "0":
                    for o, n in chunks:
                        yield o, n, None, 0
                    return
                for i in range(0, len(chunks), 2):
                    o, n = chunks[i]
                    o2, n2 = chunks[i + 1] if i + 1 < len(chunks) else (None, 0)
                    yield o, n, o2, n2

            def grp_mms(tag, blk, s, srcs, rec_s):
                """Column-tiled group reductions (den or num) + reciprocal or
                final multiply per chunk pair."""
                for o, n, o2, n2 in ndpairs():
                    nd = (psd if rec_s is not None else psn).tile(
                        [128, 512], f32, tag=tag, name=f"{tag}{blk}_{s}_{o}")
                    for m in range(4):
                        mm = MBLK[m + 1] - MBLK[m]
                        nc.tensor.matmul(nd[0:8, 0:n], gm_sb[0:mm, m, :],
                                         srcs[0:mm, m, o:o + n],
                                         start=(m == 0), stop=(m == 3),
                                         tile_position=(0, 0),
                                         skip_group_check=True)
                        if n2:
                            nc.tensor.matmul(nd[64:72, 0:n2],
                                             gm_sb[0:mm, m, :],
                                             srcs[0:mm, m, o2:o2 + n2],
                                             start=(m == 0), stop=(m == 3),
                                             tile_position=(0, 64),
                                             skip_group_check=True)
                    yield nd, o, n, o2, n2

            for blk in range(NBLK):
                R = blk * RB

                # ---- patch gather (prefetched: big DMA from DRAM) ----
                Preps = []
                for s in range(NS):
                    Prep = big.tile([128, 4, FE], bf16, tag="prep", bufs=2,
                                    name=f"Prep{blk}_{s}")
                    Preps.append(Prep)
                    for m in range(4):
                        mm = MBLK[m + 1] - MBLK[m]
                        src = bass.AP(
                            xg,
                            MBLK[m] * KR * EW + (R + s * SRB) * EW,
                            [[KR * EW, mm], [1, FE]],
                        )
                        nc.sync.dma_start(Prep[0:mm, m, :], src)

                # ---- conv0 input: im2col ----
                x36 = big.tile([36, F36], f16, tag="x36", bufs=1,
                               name=f"x36{blk}")
                for dy in range(3):
                    for dx in range(3):
                        p = (3 * dy + dx) * 4
                        src = bass.AP(
                            xs,
                            (R + dy) * XW + dx,
                            [[XR * XW, 4], [XW, ROWS0], [1, GW]],
                        )
                        nc.sync.dma_start(x36[p:p + 4, :], src)

                # ---- conv0 (per-tile shifted-duplicate copies follow) ----
                L0 = big.tile([128, F0 + 2], f16, tag="la", bufs=1,
                              name=f"L0{blk}")
                prev = 0
                for o, n in ntiles(F0):
                    ps = pscv.tile([128, 512], f32, tag="pscv",
                                   name=f"ps0_{blk}_{o}")
                    nc.tensor.matmul(ps[0:64, 0:n], w0p_sb[:, :],
                                     x36[:, o:o + n], start=True, stop=True)
                    nc.scalar.activation(L0[0:64, o:o + n], ps[0:64, 0:n],
                                         AF.Relu, bias=b03_sb[:, 0:1])
                    if o > 0:
                        nc.sync.dma_start(L0[64:128, prev:o],
                                            L0[0:64, prev + 1:o + 1])
                        prev = o
                nc.sync.dma_start(L0[64:128, prev:F0 - 1],
                                    L0[0:64, prev + 1:F0])

                # ---- conv1..conv3 ----
                Lprev = L0
                for li, (Fi, tag) in enumerate(((F1, "lb"), (F2, "la"),
                                                (F3, "lb"))):
                    Li = big.tile([128, Fi + 2], f16, tag=tag, bufs=1,
                                  name=f"L{li + 1}{blk}")
                    prev = 0
                    for o, n in ntiles(Fi):
                        ps = pscv.tile([128, 512], f32, tag="pscv",
                                       name=f"ps{li + 1}_{blk}_{o}")
                        convtile(Lprev, ps, li, o, n)
                        mrg = sm.tile([64, 512], f32, tag="mrg",
                                      name=f"mrg{li}_{blk}_{o}")
                        # ACT evacuates the right psum half (bias folded in),
                        # DVE adds the left half, ACT applies relu.
                        nc.scalar.activation(mrg[0:64, 0:n], ps[64:128, 0:n],
                                             AF.Identity,
                                             bias=b03_sb[:, li + 1:li + 2])
                        nc.vector.tensor_add(mrg[0:64, 0:n], ps[0:64, 0:n],
                                             mrg[0:64, 0:n])
                        nc.scalar.activation(Li[0:64, o:o + n], mrg[0:64, 0:n],
                                             AF.Relu)
                        if o > 0:
                            nc.sync.dma_start(Li[64:128, prev:o],
                                                Li[0:64, prev + 1:o + 1])
                            prev = o
                    nc.sync.dma_start(Li[64:128, prev:Fi - 1],
                                        Li[0:64, prev + 1:Fi])
                    Lprev = Li

                # ---- conv4 + exp -> E (bf16), sub-block at a time ----
                def conv4_sub(s):
                    E = big.tile([128, 4, FE], bf16, tag=f"e{s}", bufs=1,
                                 name=f"E{blk}_{s}")
                    for m in range(4):
                        mm = MBLK[m + 1] - MBLK[m]
                        for r in range(SRB):
                            rho = s * SRB + r
                            ps4 = pscv.tile([128, 512], f32, tag="pscv",
                                            name=f"ps4_{blk}_{rho}_{m}")
                            for dy in range(3):
                                nc.tensor.matmul(
                                    ps4[0:mm, 0:EW],
                                    w4p_sb[:, dy, MBLK[m]:MBLK[m + 1]],
                                    Lprev[0:128, (rho + dy) * GW:
                                          (rho + dy) * GW + EW],
                                    start=(dy == 0), stop=False)
                            for dy in range(3):
                                nc.tensor.matmul(
                                    ps4[0:mm, 0:EW],
                                    w4s_sb[:, dy, MBLK[m]:MBLK[m + 1]],
                                    Lprev[0:64, (rho + dy) * GW + 2:
                                          (rho + dy) * GW + 2 + EW],
                                    start=False, stop=(dy == 2))
                            nc.scalar.activation(
                                E[0:mm, m, r * EW:(r + 1) * EW],
                                ps4[0:mm, 0:EW], AF.Exp,
                                bias=b4_sb[0:mm, m:m + 1])
                    return E

                def den_sub(blk, s, E):
                    rec = sm.tile([8, FE], f32, tag="rec", bufs=2,
                                  name=f"rec{blk}_{s}")
                    for nd, o, n, o2, n2 in grp_mms("den", blk, s, E, rec):
                        nc.vector.reciprocal_approx_fast(rec[0:8, o:o + n],
                                                         nd[0:8, 0:n])
                        if n2:
                            nc.vector.reciprocal_approx_fast(
                                rec[0:8, o2:o2 + n2], nd[64:72, 0:n2])
                    return rec

                def mult_sub(s, E):
                    # E *= Prep in place, split across VectorE and GpSimdE
                    import os as _os
                    h = FE * 3 // 5
                    use_gp = _os.environ.get("GPMUL", "1") == "1"
                    for m in range(4):
                        mm = MBLK[m + 1] - MBLK[m]
                        if use_gp:
                            nc.vector.tensor_mul(E[0:mm, m, 0:h],
                                                 E[0:mm, m, 0:h],
                                                 Preps[s][0:mm, m, 0:h])
                            nc.gpsimd.tensor_mul(E[0:mm, m, h:FE],
                                                 E[0:mm, m, h:FE],
                                                 Preps[s][0:mm, m, h:FE])
                        else:
                            nc.vector.tensor_mul(E[0:mm, m, :],
                                                 E[0:mm, m, :],
                                                 Preps[s][0:mm, m, :])

                def num_sub(blk, s, E, rec):
                    for nd, o, n, o2, n2 in grp_mms("num", blk, s, E, None):
                        res = sm.tile([8, 512], f32, tag="res", bufs=3,
                                      name=f"res{blk}_{s}_{o}")
                        nc.vector.tensor_mul(res[0:8, 0:n], nd[0:8, 0:n],
                                             rec[0:8, o:o + n])
                        nc.sync.dma_start(
                            out.ap()[0:8, (R + s * SRB) * EW + o:
                                     (R + s * SRB) * EW + o + n],
                            res[0:8, 0:n])
                        if n2:
                            res2 = sm.tile([8, 512], f32, tag="res", bufs=3,
                                           name=f"res{blk}_{s}_{o2}")
                            nc.vector.tensor_mul(res2[0:8, 0:n2],
                                                 nd[64:72, 0:n2],
                                                 rec[0:8, o2:o2 + n2])
                            nc.sync.dma_start(
                                out.ap()[0:8, (R + s * SRB) * EW + o2:
                                         (R + s * SRB) * EW + o2 + n2],
                                res2[0:8, 0:n2])

                E0 = conv4_sub(0)
                rec0 = den_sub(blk, 0, E0)
                E1 = conv4_sub(1)
                mult_sub(0, E0)
                rec1 = den_sub(blk, 1, E1)
                num_sub(blk, 0, E0, rec0)
                mult_sub(1, E1)
                num_sub(blk, 1, E1, rec1)

    nc.compile()
    return nc


def _host_prep(inputs):
    mosaic = np.asarray(inputs["mosaic"], dtype=np.float32)
    gray = mosaic.sum(axis=1)                       # [2, 768, 768]
    g0 = gray[:, 0::2, 0::2]
    b_ = gray[:, 1::2, 0::2]
    r = gray[:, 0::2, 1::2]
    g1 = gray[:, 1::2, 1::2]
    x4 = np.stack([g0, b_, r, g1], axis=1)          # [2, 4, 384, 384]
    xpad = np.zeros((BS, 4, QH + 4, XW), dtype=np.float32)
    xpad[:, :, :QH, :QW] = x4

    W0 = np.asarray(inputs["W0"], np.float32)
    w0p = np.ascontiguousarray(W0.transpose(2, 3, 1, 0).reshape(36, 64))

    wp = np.empty((128, 9, 64), np.float32)
    ws = np.empty((64, 9, 64), np.float32)
    for li, wname in enumerate(("W1", "W2", "W3")):
        Wi = np.asarray(inputs[wname], np.float32)   # [64, 64, 3, 3]
        wp[0:64, 3 * li:3 * li + 3, :] = Wi[:, :, :, 0].transpose(1, 2, 0)
        wp[64:128, 3 * li:3 * li + 3, :] = Wi[:, :, :, 1].transpose(1, 2, 0)
        ws[:, 3 * li:3 * li + 3, :] = Wi[:, :, :, 2].transpose(1, 2, 0)

    W4 = np.asarray(inputs["W4"], np.float32)        # [490, 64, 3, 3]
    w4p = np.empty((128, 3, 490), np.float32)
    w4s = np.empty((64, 3, 490), np.float32)
    w4p[0:64] = W4[:, :, :, 0].transpose(1, 2, 0)
    w4p[64:128] = W4[:, :, :, 1].transpose(1, 2, 0)
    w4s[:] = W4[:, :, :, 2].transpose(1, 2, 0)

    b03 = np.stack([np.asarray(inputs[f"b{i}"], np.float32)
                    for i in range(4)], axis=1)      # [64, 4]
    b4v = np.asarray(inputs["b4"], np.float32)
    b4p = np.zeros((128, 4), np.float32)
    for c in range(490):
        b4p[c % 128, c // 128] = b4v[c]

    gmk = np.zeros((128, 4, 8), ml_dtypes.bfloat16)
    for c in range(490):
        gmk[c % 128, c // 128, CHUNK_GROUP[c // 49]] = 1

    xpad_bf = xpad.astype(ml_dtypes.bfloat16)
    w0p16 = w0p.astype(np.float16)
    wp16 = wp.astype(np.float16)
    ws16 = ws.astype(np.float16)
    w4p16 = w4p.astype(np.float16)
    w4s16 = w4s.astype(np.float16)
    in_maps = []
    for b in range(BS):
        for band in range(BANDS):
            r0 = band * 94
            slab = np.zeros((4, XR, XW), np.float16)
            hi = min(QH, r0 + XR)
            slab[:, 0:hi - r0, :] = xpad[b, :, r0:hi, :].astype(np.float16)
            # shifted-plane (im2col) tensor for the kernel-apply patches:
            # xg[49*j + 7*dy + dx, jr, jc] = plane_j[r0 + jr + 2 + dy, jc + 2 + dx]
            xgp = np.empty((490, KR, EW), ml_dtypes.bfloat16)
            for j in range(10):
                pl = xpad_bf[b, CHUNK_PLANE[j]]
                for dy in range(KS):
                    for dx in range(KS):
                        c = 49 * j + 7 * dy + dx
                        xgp[c] = pl[r0 + 2 + dy: r0 + 2 + dy + KR,
                                    2 + dx: 2 + dx + EW]
            in_maps.append({
                "xs": slab, "xg": xgp,
                "w0p": w0p16, "wp": wp16, "ws": ws16,
                "w4p": w4p16, "w4s": w4s16,
                "b03": b03, "b4": b4p, "gm": gmk,
            })
    aux = {"g0": g0, "b_": b_, "r": r, "g1": g1}
    return in_maps, aux


def _assemble(results, aux):
    full = np.empty((BS, 3, 2 * KR_TOT, 2 * KR_TOT), np.float32)
    # quarter-res computed planes [8, 374, 374] per batch
    for b in range(BS):
        qs = []
        for band in range(BANDS):
            core = b * BANDS + band
            o = results[core]["out"].reshape(8, KR, EW)
            nvalid = min(94, KR_TOT - band * 94)
            qs.append(o[:, :nvalid, :KR_TOT])
        q = np.concatenate(qs, axis=1)               # [8, 374, 374]
        crop = (slice(5, 5 + KR_TOT), slice(5, 5 + KR_TOT))
        r_pass = aux["r"][b][crop]
        b_pass = aux["b_"][b][crop]
        g0_pass = aux["g0"][b][crop]
        g1_pass = aux["g1"][b][crop]
        # red
        full[b, 0, 0::2, 0::2] = q[0]
        full[b, 0, 0::2, 1::2] = r_pass
        full[b, 0, 1::2, 0::2] = q[1]
        full[b, 0, 1::2, 1::2] = q[2]
        # green
        full[b, 1, 0::2, 0::2] = g0_pass
        full[b, 1, 0::2, 1::2] = q[6]
        full[b, 1, 1::2, 0::2] = q[7]
        full[b, 1, 1::2, 1::2] = g1_pass
        # blue
        full[b, 2, 0::2, 0::2] = q[3]
        full[b, 2, 0::2, 1::2] = q[4]
        full[b, 2, 1::2, 0::2] = b_pass
        full[b, 2, 1::2, 1::2] = q[5]
    return full


def kernel(**inputs):
    global LAST_EXEC_NS, LAST_RESULTS
    from concourse.bass_utils import run_bass_kernel_spmd

    if "nc" not in _cache:
        _cache["nc"] = _build()
    nc = _cache["nc"]

    in_maps, aux = _host_prep(inputs)
    kw = {}
    if TRACE:
        kw["trace"] = True
    res = run_bass_kernel_spmd(nc, in_maps, core_ids=list(range(8)), **kw)
    LAST_EXEC_NS = res.exec_time_ns
    LAST_RESULTS = res
    return _assemble(res.results, aux)



# revision 3
# speedup vs baseline: 1.2502x; 1.2502x over previous
"""Bayer kernel-prediction demosaic network on 8 Trainium2 NeuronCores.

Pipeline per core (one batch x one 96-row band of the 374-row quarter-res
kernel grid, with halo):
  - conv0 (4->64): host-side im2col (K=36) -> single matmul per tile
  - conv1..3 (64->64): 6 tap-matmuls (3 paired taps K=128 using a
    column-shifted duplicate of the activations in partitions 64:128,
    plus 3 single taps K=64) all accumulated into ONE psum region,
    evacuated in a single Relu+bias pass (ScalarE)
  - conv4 (64->490): same 6-matmul structure, 4 output-channel blocks,
    two output rows per psum tile; exp(.) applied on PSUM eviction
    (ScalarE) with the conv bias folded in -> E (bf16)
  - patch tensor Prep[tap_channel, pixel] gathered straight from the
    (padded) phase planes in DRAM with strided DMAs (bf16)
  - M = E * Prep on VectorE + GpSimdE
  - numerator / denominator = group-mask matmuls (bf16) contracting the
    490 tap-channels -> [8, pix] in PSUM; out = num * reciprocal(den)
All conv-chain matmuls run in fp16 with fp32 PSUM accumulation.
Host does phase extraction, weight re-layout, sharding w/ halo, and the
final pixel-shuffle assembly.
"""

import sys

sys.path.insert(0, "/opt/trn_rl_repo")

import numpy as np
import ml_dtypes

# ---------------- geometry constants ----------------
KS = 7
K2 = 49
BS = 2
H = W = 768
QH = QW = 384          # quarter-res
KR_TOT = 374           # valid kernel rows/cols
BANDS = 4              # bands per batch -> 8 cores
KR = 96                # kernel rows computed per core (94/92 valid)
RB = 16                # kernel rows per conv block
SRB = 8                # kernel rows per apply sub-block
NBLK = KR // RB
GW = 386               # conv grid width (L buffers, x36)
EW = 376               # apply/kernel grid width (374 valid + 2)
XW = 388               # padded plane width
XR = 106               # padded plane rows per band (96 + 10)
ROWS0, ROWS1, ROWS2, ROWS3 = 24, 22, 20, 18   # conv0..conv3 out rows/block
F36 = ROWS0 * GW
F0, F1, F2, F3 = ROWS0 * GW, ROWS1 * GW, ROWS2 * GW, ROWS3 * GW
FE = SRB * EW          # flat apply pixels per sub-block
OUTF = KR * EW
MBLK = [0, 128, 256, 384, 490]     # channel block boundaries
# plane (x-slab channel) feeding each 49-tap chunk of the 490 kernels:
# x channels: 0=g0 1=b 2=r 3=g1 ; chunks: 3x red, 3x blue, (g0,g1)x2
CHUNK_PLANE = [2, 2, 2, 1, 1, 1, 0, 3, 0, 3]
# 49-chunk -> output group (greens pair up)
CHUNK_GROUP = [0, 1, 2, 3, 4, 5, 6, 6, 7, 7]

TRACE = False          # set True (module attr) to profile the run
LAST_EXEC_NS = None
LAST_RESULTS = None

_cache = {}


def _build():
    import concourse.bass as bass
    import concourse.bacc as bacc
    import concourse.mybir as mybir
    import concourse.tile as tile

    f32 = mybir.dt.float32
    f16 = mybir.dt.float16
    bf16 = mybir.dt.bfloat16
    AF = mybir.ActivationFunctionType

    nc = bacc.Bacc("TRN2", target_bir_lowering=False, debug=False,
                   enable_asserts=False)

    xs36 = nc.dram_tensor("xs36", [NBLK, 36, F36], f16, kind="ExternalInput")
    xg = nc.dram_tensor("xg", [490, KR, EW], bf16, kind="ExternalInput")
    w0p = nc.dram_tensor("w0p", [36, 64], f16, kind="ExternalInput")
    wp = nc.dram_tensor("wp", [128, 9, 64], f16, kind="ExternalInput")
    ws = nc.dram_tensor("ws", [64, 9, 64], f16, kind="ExternalInput")
    w4p = nc.dram_tensor("w4p", [128, 3, 490], f16, kind="ExternalInput")
    w4s = nc.dram_tensor("w4s", [64, 3, 490], f16, kind="ExternalInput")
    b03 = nc.dram_tensor("b03", [64, 4], f32, kind="ExternalInput")
    b4 = nc.dram_tensor("b4", [128, 4], f32, kind="ExternalInput")
    gm = nc.dram_tensor("gm", [128, 4, 8], bf16, kind="ExternalInput")
    out = nc.dram_tensor("out", [8, OUTF], f32, kind="ExternalOutput")

    def ntiles(total, tsz=512):
        o = 0
        while o < total:
            n = min(tsz, total - o)
            yield o, n
            o += n

    with tile.TileContext(nc) as tc:
        with (
            tc.tile_pool(name="wts", bufs=1) as wts,
            tc.tile_pool(name="big", bufs=1) as big,
            tc.tile_pool(name="sm", bufs=2) as sm,
            tc.tile_pool(name="pscv", bufs=3, space="PSUM") as pscv,
            tc.tile_pool(name="psdn", bufs=2, space="PSUM") as psdn,
        ):
            w0p_sb = wts.tile([36, 64], f16)
            wp_sb = wts.tile([128, 9, 64], f16)
            ws_sb = wts.tile([64, 9, 64], f16)
            w4p_sb = wts.tile([128, 3, 490], f16)
            w4s_sb = wts.tile([64, 3, 490], f16)
            b03_sb = wts.tile([64, 4], f32)
            b4_sb = wts.tile([128, 4], f32)
            gm_sb = wts.tile([128, 4, 8], bf16)
            for dst, src in ((w0p_sb, w0p), (wp_sb, wp), (ws_sb, ws),
                             (w4p_sb, w4p), (w4s_sb, w4s), (b03_sb, b03),
                             (b4_sb, b4), (gm_sb, gm)):
                nc.sync.dma_start(dst[:], src.ap())

            NS = RB // SRB

            def conv_chunk(Lprev, ps, li, o, n):
                """conv1..3 chunk [o, o+n): 6 tap matmuls, all accumulated
                into ps[0:64, 0:n] (single psum region, single-pass evac)."""
                for h0, hn in ntiles(n):
                    for dy in range(3):
                        nc.tensor.matmul(
                            ps[0:64, h0:h0 + hn],
                            wp_sb[:, 3 * li + dy, :],
                            Lprev[0:128, o + h0 + dy * GW:
                                  o + h0 + dy * GW + hn],
                            start=(dy == 0), stop=False)
                    for dy in range(3):
                        nc.tensor.matmul(
                            ps[0:64, h0:h0 + hn],
                            ws_sb[0:64, 3 * li + dy, :],
                            Lprev[0:64, o + h0 + dy * GW + 2:
                                  o + h0 + dy * GW + 2 + hn],
                            start=False, stop=(dy == 2))

            def x36_load(blk, x36t):
                src = bass.AP(xs36, blk * 36 * F36, [[F36, 36], [1, F36]])
                nc.scalar.dma_start(x36t[:], src)

            x36_tiles = {}

            def get_x36(blk):
                if blk not in x36_tiles:
                    x36t = big.tile([36, F36], f16, tag="x36", bufs=2,
                                    name=f"x36{blk}")
                    x36_tiles[blk] = x36t
                    x36_load(blk, x36t)
                return x36_tiles[blk]

            CH = 1024          # psum chunk (2 banks)

            for blk in range(NBLK):
                R = blk * RB

                # ---- patch gather (prefetched: big DMA from DRAM) ----
                Preps = []
                for s in range(NS):
                    Prep = big.tile([128, 4, FE], bf16, tag="prep", bufs=2,
                                    name=f"Prep{blk}_{s}")
                    Preps.append(Prep)
                    for m in range(4):
                        mm = MBLK[m + 1] - MBLK[m]
                        src = bass.AP(
                            xg,
                            MBLK[m] * KR * EW + (R + s * SRB) * EW,
                            [[KR * EW, mm], [1, FE]],
                        )
                        nc.sync.dma_start(Prep[0:mm, m, :], src)

                # ---- conv0 input: host-side im2col, one DMA ----
                x36 = get_x36(blk)

                # ---- conv0 ----
                L0 = big.tile([128, F0 + 2], f16, tag="la", bufs=1,
                              name=f"L0{blk}")
                prev = 0
                for o, n in ntiles(F0, CH):
                    ps = pscv.tile([128, CH], f32, tag="pscv",
                                   name=f"ps0_{blk}_{o}")
                    for h0, hn in ntiles(n):
                        nc.tensor.matmul(ps[0:64, h0:h0 + hn], w0p_sb[:, :],
                                         x36[:, o + h0:o + h0 + hn],
                                         start=True, stop=True)
                    nc.scalar.activation(L0[0:64, o:o + n], ps[0:64, 0:n],
                                         AF.Relu, bias=b03_sb[:, 0:1])
                    if o > 0:
                        nc.scalar.dma_start(L0[64:128, prev:o],
                                            L0[0:64, prev + 1:o + 1])
                        prev = o
                nc.scalar.dma_start(L0[64:128, prev:F0 - 1],
                                    L0[0:64, prev + 1:F0])
                # prefetch next block's x36 while this block computes
                if blk + 1 < NBLK:
                    get_x36(blk + 1)

                # ---- conv1..conv3 ----
                Lprev = L0
                for li, (Fi, tag) in enumerate(((F1, "lb"), (F2, "la"),
                                                (F3, "lb"))):
                    Li = big.tile([128, Fi + 2], f16, tag=tag, bufs=1,
                                  name=f"L{li + 1}{blk}")
                    prev = 0
                    for o, n in ntiles(Fi, CH):
                        ps = pscv.tile([128, CH], f32, tag="pscv",
                                       name=f"ps{li + 1}_{blk}_{o}")
                        conv_chunk(Lprev, ps, li, o, n)
                        nc.scalar.activation(Li[0:64, o:o + n], ps[0:64, 0:n],
                                             AF.Relu,
                                             bias=b03_sb[:, li + 1:li + 2])
                        if o > 0:
                            nc.scalar.dma_start(Li[64:128, prev:o],
                                                Li[0:64, prev + 1:o + 1])
                            prev = o
                    nc.scalar.dma_start(Li[64:128, prev:Fi - 1],
                                        Li[0:64, prev + 1:Fi])
                    Lprev = Li

                # ---- conv4 + exp -> E (bf16), 2 rows per psum tile ----
                def conv4_sub(s):
                    E = big.tile([128, 4, FE], bf16, tag=f"e{s}", bufs=1,
                                 name=f"E{blk}_{s}")
                    for m in range(4):
                        mm = MBLK[m + 1] - MBLK[m]
                        for r in range(0, SRB, 2):
                            ps4 = pscv.tile([128, 2, 512], f32, tag="pscv",
                                            name=f"ps4_{blk}_{s}_{m}_{r}")
                            for rr in range(2):
                                rho = s * SRB + r + rr
                                for dy in range(3):
                                    nc.tensor.matmul(
                                        ps4[0:mm, rr, 0:EW],
                                        w4p_sb[:, dy, MBLK[m]:MBLK[m + 1]],
                                        Lprev[0:128, (rho + dy) * GW:
                                              (rho + dy) * GW + EW],
                                        start=(dy == 0), stop=False)
                                for dy in range(3):
                                    nc.tensor.matmul(
                                        ps4[0:mm, rr, 0:EW],
                                        w4s_sb[0:64, dy,
                                               MBLK[m]:MBLK[m + 1]],
                                        Lprev[0:64, (rho + dy) * GW + 2:
                                              (rho + dy) * GW + 2 + EW],
                                        start=False, stop=(dy == 2))
                            nc.scalar.activation(
                                E[0:mm, m, (r * EW):(r + 2) * EW],
                                ps4[0:mm, 0:2, 0:EW], AF.Exp,
                                bias=b4_sb[0:mm, m:m + 1])
                    return E

                def den_sub(blk, s, E):
                    rec = sm.tile([8, FE], f32, tag="rec", bufs=2,
                                  name=f"rec{blk}_{s}")
                    for o, n in ntiles(FE):
                        nd = psdn.tile([128, 512], f32, tag="psdn",
                                       name=f"den{blk}_{s}_{o}")
                        for m in range(4):
                            mm = MBLK[m + 1] - MBLK[m]
                            nc.tensor.matmul(nd[0:8, 0:n], gm_sb[0:mm, m, :],
                                             E[0:mm, m, o:o + n],
                                             start=(m == 0), stop=(m == 3))
                        nc.vector.reciprocal_approx_fast(rec[0:8, o:o + n],
                                                         nd[0:8, 0:n])
                    return rec

                def mult_sub(s, E):
                    # E *= Prep in place, split across VectorE and GpSimdE
                    h = FE * 3 // 5
                    for m in range(4):
                        mm = MBLK[m + 1] - MBLK[m]
                        nc.vector.tensor_mul(E[0:mm, m, 0:h],
                                             E[0:mm, m, 0:h],
                                             Preps[s][0:mm, m, 0:h])
                        nc.gpsimd.tensor_mul(E[0:mm, m, h:FE],
                                             E[0:mm, m, h:FE],
                                             Preps[s][0:mm, m, h:FE])

                def num_sub(blk, s, E, rec):
                    for o, n in ntiles(FE):
                        nd = psdn.tile([128, 512], f32, tag="psdn",
                                       name=f"num{blk}_{s}_{o}")
                        for m in range(4):
                            mm = MBLK[m + 1] - MBLK[m]
                            nc.tensor.matmul(nd[0:8, 0:n], gm_sb[0:mm, m, :],
                                             E[0:mm, m, o:o + n],
                                             start=(m == 0), stop=(m == 3))
                        res = sm.tile([8, 512], f32, tag="res", bufs=3,
                                      name=f"res{blk}_{s}_{o}")
                        nc.vector.tensor_mul(res[0:8, 0:n], nd[0:8, 0:n],
                                             rec[0:8, o:o + n])
                        nc.sync.dma_start(
                            out.ap()[0:8, (R + s * SRB) * EW + o:
                                     (R + s * SRB) * EW + o + n],
                            res[0:8, 0:n])

                E0 = conv4_sub(0)
                rec0 = den_sub(blk, 0, E0)
                E1 = conv4_sub(1)
                mult_sub(0, E0)
                rec1 = den_sub(blk, 1, E1)
                num_sub(blk, 0, E0, rec0)
                mult_sub(1, E1)
                num_sub(blk, 1, E1, rec1)

    nc.compile()
    return nc


def _host_prep(inputs):
    mosaic = np.asarray(inputs["mosaic"], dtype=np.float32)
    gray = mosaic.sum(axis=1)                       # [2, 768, 768]
    g0 = gray[:, 0::2, 0::2]
    b_ = gray[:, 1::2, 0::2]
    r = gray[:, 0::2, 1::2]
    g1 = gray[:, 1::2, 1::2]
    x4 = np.stack([g0, b_, r, g1], axis=1)          # [2, 4, 384, 384]
    xpad = np.zeros((BS, 4, QH + 4, XW), dtype=np.float32)
    xpad[:, :, :QH, :QW] = x4

    W0 = np.asarray(inputs["W0"], np.float32)
    w0p = np.ascontiguousarray(W0.transpose(2, 3, 1, 0).reshape(36, 64))

    wp = np.empty((128, 9, 64), np.float32)
    ws = np.empty((64, 9, 64), np.float32)
    for li, wname in enumerate(("W1", "W2", "W3")):
        Wi = np.asarray(inputs[wname], np.float32)   # [64, 64, 3, 3]
        wp[0:64, 3 * li:3 * li + 3, :] = Wi[:, :, :, 0].transpose(1, 2, 0)
        wp[64:128, 3 * li:3 * li + 3, :] = Wi[:, :, :, 1].transpose(1, 2, 0)
        ws[:, 3 * li:3 * li + 3, :] = Wi[:, :, :, 2].transpose(1, 2, 0)

    W4 = np.asarray(inputs["W4"], np.float32)        # [490, 64, 3, 3]
    w4p = np.empty((128, 3, 490), np.float32)
    w4s = np.empty((64, 3, 490), np.float32)
    w4p[0:64] = W4[:, :, :, 0].transpose(1, 2, 0)
    w4p[64:128] = W4[:, :, :, 1].transpose(1, 2, 0)
    w4s[:] = W4[:, :, :, 2].transpose(1, 2, 0)

    b03 = np.stack([np.asarray(inputs[f"b{i}"], np.float32)
                    for i in range(4)], axis=1)      # [64, 4]
    b4v = np.asarray(inputs["b4"], np.float32)
    b4p = np.zeros((128, 4), np.float32)
    for c in range(490):
        b4p[c % 128, c // 128] = b4v[c]

    gmk = np.zeros((128, 4, 8), ml_dtypes.bfloat16)
    for c in range(490):
        gmk[c % 128, c // 128, CHUNK_GROUP[c // 49]] = 1

    xpad_bf = xpad.astype(ml_dtypes.bfloat16)
    xpad16 = xpad.astype(np.float16)
    w0p16 = w0p.astype(np.float16)
    wp16 = wp.astype(np.float16)
    ws16 = ws.astype(np.float16)
    w4p16 = w4p.astype(np.float16)
    w4s16 = w4s.astype(np.float16)
    in_maps = []
    for b in range(BS):
        for band in range(BANDS):
            r0 = band * 94
            # host-side im2col for conv0:
            # xs36[blk, (3dy+dx)*4+c, row*GW+col]
            #   = xpad[b, c, r0 + blk*RB + dy + row, dx + col]
            xs36 = np.empty((NBLK, 36, F36), np.float16)
            for blk in range(NBLK):
                R = blk * RB
                for dy in range(3):
                    for dx in range(3):
                        for c in range(4):
                            p = (3 * dy + dx) * 4 + c
                            xs36[blk, p] = xpad16[
                                b, c, r0 + R + dy:r0 + R + dy + ROWS0,
                                dx:dx + GW].reshape(-1)
            # shifted-plane (im2col) tensor for the kernel-apply patches:
            # xg[49*j + 7*dy + dx, jr, jc] = plane_j[r0 + jr + 2 + dy, jc + 2 + dx]
            xgp = np.empty((490, KR, EW), ml_dtypes.bfloat16)
            for j in range(10):
                pl = xpad_bf[b, CHUNK_PLANE[j]]
                for dy in range(KS):
                    for dx in range(KS):
                        c = 49 * j + 7 * dy + dx
                        xgp[c] = pl[r0 + 2 + dy: r0 + 2 + dy + KR,
                                    2 + dx: 2 + dx + EW]
            in_maps.append({
                "xs36": xs36, "xg": xgp,
                "w0p": w0p16, "wp": wp16, "ws": ws16,
                "w4p": w4p16, "w4s": w4s16,
                "b03": b03, "b4": b4p, "gm": gmk,
            })
    aux = {"g0": g0, "b_": b_, "r": r, "g1": g1}
    return in_maps, aux


def _assemble(results, aux):
    full = np.empty((BS, 3, 2 * KR_TOT, 2 * KR_TOT), np.float32)
    # quarter-res computed planes [8, 374, 374] per batch
    for b in range(BS):
        qs = []
        for band in range(BANDS):
            core = b * BANDS + band
            o = results[core]["out"].reshape(8, KR, EW)
            nvalid = min(94, KR_TOT - band * 94)
            qs.append(o[:, :nvalid, :KR_TOT])
        q = np.concatenate(qs, axis=1)               # [8, 374, 374]
        crop = (slice(5, 5 + KR_TOT), slice(5, 5 + KR_TOT))
        r_pass = aux["r"][b][crop]
        b_pass = aux["b_"][b][crop]
        g0_pass = aux["g0"][b][crop]
        g1_pass = aux["g1"][b][crop]
        # red
        full[b, 0, 0::2, 0::2] = q[0]
        full[b, 0, 0::2, 1::2] = r_pass
        full[b, 0, 1::2, 0::2] = q[1]
        full[b, 0, 1::2, 1::2] = q[2]
        # green
        full[b, 1, 0::2, 0::2] = g0_pass
        full[b, 1, 0::2, 1::2] = q[6]
        full[b, 1, 1::2, 0::2] = q[7]
        full[b, 1, 1::2, 1::2] = g1_pass
        # blue
        full[b, 2, 0::2, 0::2] = q[3]
        full[b, 2, 0::2, 1::2] = q[4]
        full[b, 2, 1::2, 0::2] = b_pass
        full[b, 2, 1::2, 1::2] = q[5]
    return full


def kernel(**inputs):
    global LAST_EXEC_NS, LAST_RESULTS
    from concourse.bass_utils import run_bass_kernel_spmd

    if "nc" not in _cache:
        _cache["nc"] = _build()
    nc = _cache["nc"]

    in_maps, aux = _host_prep(inputs)
    kw = {}
    if TRACE:
        kw["trace"] = True
    res = run_bass_kernel_spmd(nc, in_maps, core_ids=list(range(8)), **kw)
    LAST_EXEC_NS = res.exec_time_ns
    LAST_RESULTS = res
    return _assemble(res.results, aux)
